# revision 1
# baseline (speedup 1.0000x reference)
"""CustomGATLayerEdgeReprFeat on 8 TRN2 NeuronCores (Bass/Tile).

Strategy (dst-sorted stripe sharding):
  - Host folds the weights and precomputes per-node tables from h:
      tsrc[n] = [p_src(128) | a_src(8) | z(128)]   (gathered per edge via src)
      tdst[n] = [p_dst + bproj (128) | a_dst(8)]   (read contiguously per stripe)
    where z = einsum(h, Wh), p_* are z @ Wproj-blocks, a_* are z . Wattn-blocks.
    With these, per edge:
      attn  = leaky_relu(a_e + a_src[src] + a_dst[dst]),  a_e = e @ A_e
      eproj = p_e + p_src[src] + p_dst[dst],              p_e = e @ M_e
    and the segment softmax needs no max-subtraction (attn is O(1)-bounded, so
    exp() is safe and alpha is mathematically unchanged; empty segments give
    denom=0 -> h_agg=0, matching the reference's isfinite guard).
  - Edges are sorted by dst and grouped into 128-node stripes; each stripe is
    padded to a uniform B = BT*128 edges so one SPMD program serves all cores.
    Core c owns S consecutive stripes (S*128 nodes) and exactly their edges.
  - Per 128-edge tile on device: indirect-gather tsrc rows; PE transposes e
    and accumulates e@[M_e|A_e] + sel@tdst + I@srow in one PSUM tile; DVE/ACT
    compute ex=exp(leaky(attn)) and e_out = e + elu(eproj); the segment sum is
    a PE matmul agg += sel.T @ [z*ex | ex] accumulated per-stripe in PSUM
    (sel[e,n] = (dst_rel[e]==n) built with iota + is_equal; pad edges get
    dst_rel=255 so they contribute nothing and their e_out rows are dropped).
  - Per stripe flush: h_out = h + elu(agg/denominator), written contiguously.
  - Host unpermutes e_out back to the original edge order.
"""
import math
import numpy as np

import concourse.bass as bass
import concourse.bacc as bacc
import concourse.tile as tile
from concourse import mybir
from concourse.masks import make_identity
from concourse.bass_utils import run_bass_kernel_spmd

F32 = mybir.dt.float32
I32 = mybir.dt.int32
P = 128
H = 8
O = 16
D_SROW = 264   # p_src(128) | a_src(8) | z(128)
D_ROW = 136    # 128 + 8
N_CORES = 8

_NC_CACHE = {}


def build_nc(S, BT, NPAD, num_devices=N_CORES):
    key = (S, BT, NPAD, num_devices)
    if key in _NC_CACHE:
        return _NC_CACHE[key]
    B = BT * P
    nc = bacc.Bacc("TRN2", target_bir_lowering=False, debug=False,
                   num_devices=num_devices)
    e_sp = nc.dram_tensor("e_sp", [S * P, B], F32, kind="ExternalInput")
    tsrc = nc.dram_tensor("tsrc", [NPAD, D_SROW], F32, kind="ExternalInput")
    tdst = nc.dram_tensor("tdst", [S * P, D_ROW], F32, kind="ExternalInput")
    hsh = nc.dram_tensor("hsh", [S * P, 128], F32, kind="ExternalInput")
    srci = nc.dram_tensor("srci", [S * P, BT], I32, kind="ExternalInput")
    dstr = nc.dram_tensor("dstr", [S * P, BT], I32, kind="ExternalInput")
    meae = nc.dram_tensor("meae", [128, D_ROW], F32, kind="ExternalInput")
    eo = nc.dram_tensor("eo", [S * P, B], F32, kind="ExternalOutput")
    ho = nc.dram_tensor("ho", [S * P, 128], F32, kind="ExternalOutput")

    AD = mybir.AluOpType.add
    MX = mybir.AluOpType.max
    EQ = mybir.AluOpType.is_equal
    EXP = mybir.ActivationFunctionType.Exp
    RELU = mybir.ActivationFunctionType.Relu

    with tile.TileContext(nc) as tc:
        with (
            tc.tile_pool(name="const", bufs=1) as cp,
            tc.tile_pool(name="stripe", bufs=2) as stp,
            tc.tile_pool(name="sb", bufs=3) as sb,
            tc.tile_pool(name="ps", bufs=2, space="PSUM") as ps,
            tc.tile_pool(name="aggp", bufs=2, space="PSUM") as aggp,
        ):
            ident = cp.tile([P, P], F32)
            make_identity(nc, ident[:])
            iotar_i = cp.tile([P, P], I32)
            nc.gpsimd.iota(iotar_i[:], pattern=[[1, P]], base=0, channel_multiplier=0)
            iotar = cp.tile([P, P], F32)
            nc.vector.tensor_copy(iotar[:], iotar_i[:])
            iotac_i = cp.tile([P, P], I32)
            nc.gpsimd.iota(iotac_i[:], pattern=[[0, P]], base=0, channel_multiplier=1)
            iotac = cp.tile([P, P], F32)
            nc.vector.tensor_copy(iotac[:], iotac_i[:])
            meae_t = cp.tile([128, D_ROW], F32)
            nc.sync.dma_start(out=meae_t[:], in_=meae[:])

            for s in range(S):
                r0 = s * P
                agg = aggp.tile([P, D_ROW], F32, tag="agg")
                tdst_t = stp.tile([P, D_ROW], F32, tag="tdst")
                nc.sync.dma_start(out=tdst_t[:], in_=tdst[r0:r0 + P, :])
                e_st = stp.tile([P, B], F32, tag="e_st")
                nc.sync.dma_start(out=e_st[:], in_=e_sp[r0:r0 + P, :])
                si_all = stp.tile([P, BT], I32, tag="si_all")
                nc.sync.dma_start(out=si_all[:], in_=srci[r0:r0 + P, :])
                dri_all = stp.tile([P, BT], I32, tag="dri_all")
                nc.sync.dma_start(out=dri_all[:], in_=dstr[r0:r0 + P, :])
                dr_all = stp.tile([P, BT], F32, tag="dr_all")
                nc.vector.tensor_copy(dr_all[:], dri_all[:])
                eo_st = stp.tile([P, B], F32, tag="eo_st")

                for bt in range(BT):
                    ec = slice(bt * P, (bt + 1) * P)
                    srow = sb.tile([P, D_SROW], F32, tag="srow")
                    nc.gpsimd.indirect_dma_start(
                        out=srow[:], out_offset=None, in_=tsrc[:],
                        in_offset=bass.IndirectOffsetOnAxis(
                            ap=si_all[:, bt:bt + 1], axis=0))

                    # sel[e,n] / selT[n,e] equality matrices
                    dcol = dr_all[:, bt:bt + 1]
                    dT_ps = ps.tile([P, P], F32, tag="dTp")
                    nc.tensor.transpose(out=dT_ps[:], in_=dcol.to_broadcast([P, P]),
                                        identity=ident[:])
                    selT = sb.tile([P, P], F32, tag="selT")
                    nc.vector.tensor_tensor(out=selT[:], in0=dT_ps[:], in1=iotac[:], op=EQ)
                    sel = sb.tile([P, P], F32, tag="sel")
                    nc.vector.tensor_tensor(out=sel[:], in0=dcol.to_broadcast([P, P]),
                                            in1=iotar[:], op=EQ)

                    # pa = e@[M_e|A_e] + sel@tdst + srow[:,0:136], all in PSUM
                    eT_ps = ps.tile([P, 128], F32, tag="eTp")
                    nc.tensor.transpose(out=eT_ps[:], in_=e_st[:, ec], identity=ident[:])
                    eT = sb.tile([P, 128], F32, tag="eT")
                    nc.vector.tensor_copy(eT[:], eT_ps[:])
                    pa_ps = ps.tile([P, D_ROW], F32, tag="pap")
                    nc.tensor.matmul(pa_ps[:], eT[:], meae_t[:], start=True, stop=False)
                    nc.tensor.matmul(pa_ps[:], selT[:], tdst_t[:], start=False, stop=False)
                    nc.tensor.matmul(pa_ps[:], ident[:], srow[:, 0:D_ROW],
                                     start=False, stop=True)

                    # ex = exp(leaky_relu(attn)) into rhs[:,128:136]
                    rhs = sb.tile([P, D_ROW], F32, tag="rhs")
                    lk = sb.tile([P, H], F32, tag="lk")
                    nc.vector.tensor_scalar_mul(lk[:], pa_ps[:, 128:136], 0.01)
                    lk2 = sb.tile([P, H], F32, tag="lk2")
                    nc.vector.tensor_tensor(out=lk2[:], in0=pa_ps[:, 128:136],
                                            in1=lk[:], op=MX)
                    nc.scalar.activation(rhs[:, 128:136], lk2[:], EXP)

                    # e_out slice = e + elu(e_proj); elu(x) = exp(-relu(-x)) - 1 + relu(x)
                    rn = sb.tile([P, 128], F32, tag="rn")
                    nc.scalar.activation(rn[:], pa_ps[:, :128], RELU, scale=-1.0)
                    exel = sb.tile([P, 128], F32, tag="exel")
                    nc.scalar.activation(exel[:], rn[:], EXP, scale=-1.0)
                    rp = sb.tile([P, 128], F32, tag="rp")
                    nc.scalar.activation(rp[:], pa_ps[:, :128], RELU)
                    d1 = sb.tile([P, 128], F32, tag="d1")
                    nc.vector.tensor_tensor(out=d1[:], in0=exel[:], in1=rp[:], op=AD)
                    d2 = sb.tile([P, 128], F32, tag="d2")
                    nc.vector.tensor_tensor(out=d2[:], in0=d1[:], in1=e_st[:, ec], op=AD)
                    nc.vector.tensor_scalar_add(eo_st[:, ec], d2[:], -1.0)

                    # rhs[:,0:128] = z * ex (broadcast ex over each head's 16 cols)
                    nc.vector.tensor_tensor(
                        out=rhs[:, 0:128].rearrange("p (h o) -> p h o", h=H),
                        in0=srow[:, D_ROW:D_SROW].rearrange("p (h o) -> p h o", h=H),
                        in1=rhs[:, 128:136].to_broadcast([P, H, O]),
                        op=mybir.AluOpType.mult)
                    # agg += sel.T @ [z*ex | ex]
                    nc.tensor.matmul(agg[:], sel[:], rhs[:],
                                     start=(bt == 0), stop=(bt == BT - 1))

                nc.sync.dma_start(out=eo[r0:r0 + P, :], in_=eo_st[:])

                # stripe flush: h_out = h + elu(agg/denom)
                h_t = sb.tile([P, 128], F32, tag="h_t")
                nc.sync.dma_start(out=h_t[:], in_=hsh[r0:r0 + P, :])
                den = sb.tile([P, H], F32, tag="den")
                nc.vector.tensor_scalar_max(den[:], agg[:, 128:136], 1e-9)
                rec = sb.tile([P, H], F32, tag="rec")
                nc.vector.reciprocal(rec[:], den[:])
                hag = sb.tile([P, 128], F32, tag="hag")
                nc.vector.tensor_tensor(
                    out=hag[:].rearrange("p (h o) -> p h o", h=H),
                    in0=agg[:, 0:128].rearrange("p (h o) -> p h o", h=H),
                    in1=rec[:].to_broadcast([P, H, O]),
                    op=mybir.AluOpType.mult)
                rn2 = sb.tile([P, 128], F32, tag="rn2")
                nc.scalar.activation(rn2[:], hag[:], RELU, scale=-1.0)
                exel2 = sb.tile([P, 128], F32, tag="exel2")
                nc.scalar.activation(exel2[:], rn2[:], EXP, scale=-1.0)
                rp2 = sb.tile([P, 128], F32, tag="rp2")
                nc.scalar.activation(rp2[:], hag[:], RELU)
                el2 = sb.tile([P, 128], F32, tag="el2")
                nc.vector.tensor_tensor(out=el2[:], in0=exel2[:], in1=rp2[:], op=AD)
                d3 = sb.tile([P, 128], F32, tag="d3")
                nc.vector.tensor_tensor(out=d3[:], in0=el2[:], in1=h_t[:], op=AD)
                ho_t = sb.tile([P, 128], F32, tag="ho_t")
                nc.vector.tensor_scalar_add(ho_t[:], d3[:], -1.0)
                nc.sync.dma_start(out=ho[r0:r0 + P, :], in_=ho_t[:])

    nc.compile()
    _NC_CACHE[key] = nc
    return nc


def prep(h, e, src, dst, Wh, We, Wproj, bproj, Wattn, n_cores=N_CORES):
    """Host-side fold + sort + shard. Returns (in_maps, meta)."""
    N, E = h.shape[0], e.shape[0]
    h = np.ascontiguousarray(np.asarray(h, np.float32))
    e = np.ascontiguousarray(np.asarray(e, np.float32))
    src = np.asarray(src, np.int32); dst = np.asarray(dst, np.int32)
    Wh = np.asarray(Wh, np.float32); We = np.asarray(We, np.float32)
    Wproj = np.asarray(Wproj, np.float32); bproj = np.asarray(bproj, np.float32)
    Wattn = np.asarray(Wattn, np.float32)

    S = math.ceil(N / (P * n_cores))          # stripes per core
    nstr = S * n_cores
    NPAD = nstr * P

    # ---- weight folding + node tables ----
    z = (h @ Wh.transpose(1, 0, 2).reshape(128, 128)).reshape(N, H, O)
    M_e = np.einsum('hik,hko->iho', We, Wproj[:, :O, :]).reshape(128, 128)
    A_e = np.einsum('hik,hk->ih', We, Wattn[:, :O])
    meae = np.concatenate([M_e, A_e], axis=1).astype(np.float32)
    p_src = np.einsum('nhk,hko->nho', z, Wproj[:, O:2 * O, :]).reshape(N, 128)
    a_src = np.einsum('nhk,hk->nh', z, Wattn[:, O:2 * O])
    p_dst = (np.einsum('nhk,hko->nho', z, Wproj[:, 2 * O:, :]) + bproj).reshape(N, 128)
    a_dst = np.einsum('nhk,hk->nh', z, Wattn[:, 2 * O:])
    tsrc = np.zeros((NPAD, D_SROW), np.float32)
    tsrc[:N] = np.concatenate([p_src, a_src, z.reshape(N, 128)], axis=1)
    tdst_full = np.zeros((NPAD, D_ROW), np.float32)
    tdst_full[:N] = np.concatenate([p_dst, a_dst], axis=1)
    hpad = np.zeros((NPAD, 128), np.float32)
    hpad[:N] = h

    # ---- sort edges by dst, stripe-pad to uniform B = BT*128 ----
    perm = np.argsort(dst, kind='stable').astype(np.int64)
    dst_s = dst[perm]
    starts = np.searchsorted(dst_s, np.arange(nstr) * P).astype(np.int64)
    ends = np.searchsorted(dst_s, np.arange(nstr) * P + P).astype(np.int64)
    cnt = ends - starts
    BT = max(1, math.ceil(cnt.max() / P))
    B = BT * P
    slot = np.arange(B)
    mat = starts[:, None] + slot[None, :]
    valid = slot[None, :] < cnt[:, None]                   # [nstr, B]
    spos = np.where(valid, mat, 0)
    eidx = perm[spos]                                      # original edge ids
    eidx_g = np.where(valid, eidx, 0)
    src_pad = np.where(valid, src[eidx_g], 0).astype(np.int32)
    dstrel_pad = np.where(valid, dst[eidx_g] - (np.arange(nstr) * P)[:, None],
                          255).astype(np.int32)
    e_sp = e[eidx_g.reshape(-1)]
    # block-transpose: [nstr, BT, 128e, 128k] -> [nstr*128e, BT*128k]
    e_sp = np.ascontiguousarray(
        e_sp.reshape(nstr, BT, P, 128).transpose(0, 2, 1, 3).reshape(nstr * P, B))
    src_pad = np.ascontiguousarray(
        src_pad.reshape(nstr, BT, P).transpose(0, 2, 1).reshape(nstr * P, BT))
    dstrel_pad = np.ascontiguousarray(
        dstrel_pad.reshape(nstr, BT, P).transpose(0, 2, 1).reshape(nstr * P, BT))

    in_maps = []
    for c in range(n_cores):
        lo, hi = c * S * P, (c + 1) * S * P
        in_maps.append({
            "e_sp": e_sp[lo:hi],
            "tsrc": tsrc,
            "tdst": tdst_full[lo:hi],
            "hsh": hpad[lo:hi],
            "srci": src_pad[lo:hi],
            "dstr": dstrel_pad[lo:hi],
            "meae": meae,
        })
    meta = dict(S=S, BT=BT, NPAD=NPAD, N=N, E=E, n_cores=n_cores,
                eidx=eidx, valid=valid)
    return in_maps, meta


def unshard(results, meta):
    N, E, BT = meta["N"], meta["E"], meta["BT"]
    nstr = meta["NPAD"] // P
    ho = np.concatenate([r["ho"] for r in results], axis=0)[:N]
    eo = np.concatenate([r["eo"] for r in results], axis=0)
    eo = eo.reshape(nstr, P, BT, 128).transpose(0, 2, 1, 3).reshape(nstr, BT * P, 128)
    e_out = np.empty((E, 128), np.float32)
    v = meta["valid"]
    e_out[meta["eidx"][v]] = eo[v]
    return np.ascontiguousarray(ho), e_out


def kernel(h, e, src, dst, Wh, We, Wproj, bproj, Wattn):
    in_maps, meta = prep(h, e, src, dst, Wh, We, Wproj, bproj, Wattn)
    nc = build_nc(meta["S"], meta["BT"], meta["NPAD"])
    res = run_bass_kernel_spmd(nc, in_maps, core_ids=list(range(N_CORES)))
    h_out, e_out = unshard(res.results, meta)
    return h_out, e_out


# revision 2
# speedup vs baseline: 1.5650x; 1.5650x over previous
"""CustomGATLayerEdgeReprFeat on 8 TRN2 NeuronCores (Bass/Tile).

Strategy (dst-sorted stripe sharding):
  - Host folds the weights and precomputes per-node tables from h:
      tsrc[n] = [p_src(128) | a_src(8) | z(128)]   (gathered per edge via src)
      tdst[n] = [p_dst + bproj (128) | a_dst(8)]   (read contiguously per stripe)
    where z = einsum(h, Wh), p_* are z @ Wproj-blocks, a_* are z . Wattn-blocks.
    With these, per edge:
      attn  = leaky_relu(a_e + a_src[src] + a_dst[dst]),  a_e = e @ A_e
      eproj = p_e + p_src[src] + p_dst[dst],              p_e = e @ M_e
    and the segment softmax needs no max-subtraction (attn is O(1)-bounded, so
    exp() is safe and alpha is mathematically unchanged; empty segments give
    denom=0 -> h_agg=0, matching the reference's isfinite guard).
  - Edges are sorted by dst and grouped into 128-node stripes; each stripe is
    padded to a uniform B = BT*128 edges so one SPMD program serves all cores.
    Core c owns S consecutive stripes (S*128 nodes) and exactly their edges.
  - Per 128-edge tile on device: indirect-gather tsrc rows; PE transposes e
    and accumulates e@[M_e|A_e] + sel@tdst + I@srow in one PSUM tile; DVE/ACT
    compute ex=exp(leaky(attn)) and e_out = e + elu(eproj); the segment sum is
    a PE matmul agg += sel.T @ [z*ex | ex] accumulated per-stripe in PSUM
    (sel[e,n] = (dst_rel[e]==n) built with iota + is_equal; pad edges get
    dst_rel=255 so they contribute nothing and their e_out rows are dropped).
  - Per stripe flush: h_out = h + elu(agg/denominator), written contiguously.
  - Host unpermutes e_out back to the original edge order.
"""
import math
import numpy as np

import concourse.bass as bass
import concourse.bacc as bacc
import concourse.tile as tile
from concourse import mybir
from concourse.masks import make_identity
from concourse.bass_utils import run_bass_kernel_spmd

F32 = mybir.dt.float32
I32 = mybir.dt.int32
P = 128
H = 8
O = 16
D_SROW = 264   # p_src(128) | a_src(8) | z(128)
D_ROW = 136    # 128 + 8
N_CORES = 8

_NC_CACHE = {}


def build_nc(S, BT, NPAD, num_devices=N_CORES):
    key = (S, BT, NPAD, num_devices)
    if key in _NC_CACHE:
        return _NC_CACHE[key]
    B = BT * P
    nc = bacc.Bacc("TRN2", target_bir_lowering=False, debug=False,
                   num_devices=num_devices)
    e_sp = nc.dram_tensor("e_sp", [S * P, B], F32, kind="ExternalInput")
    tsrc = nc.dram_tensor("tsrc", [NPAD, D_SROW], F32, kind="ExternalInput")
    tdst = nc.dram_tensor("tdst", [S * P, D_ROW], F32, kind="ExternalInput")
    hsh = nc.dram_tensor("hsh", [S * P, 128], F32, kind="ExternalInput")
    srci = nc.dram_tensor("srci", [S * P, BT], I32, kind="ExternalInput")
    dstr = nc.dram_tensor("dstr", [S * P, BT], I32, kind="ExternalInput")
    meae = nc.dram_tensor("meae", [128, D_ROW], F32, kind="ExternalInput")
    eo = nc.dram_tensor("eo", [S * P, B], F32, kind="ExternalOutput")
    ho = nc.dram_tensor("ho", [S * P, 128], F32, kind="ExternalOutput")

    AD = mybir.AluOpType.add
    MX = mybir.AluOpType.max
    EQ = mybir.AluOpType.is_equal
    EXP = mybir.ActivationFunctionType.Exp
    RELU = mybir.ActivationFunctionType.Relu

    with tile.TileContext(nc) as tc:
        with (
            tc.tile_pool(name="const", bufs=1) as cp,
            tc.tile_pool(name="stripe", bufs=2) as stp,
            tc.tile_pool(name="sb", bufs=4) as sb,
            tc.tile_pool(name="ps", bufs=2, space="PSUM") as ps,
            tc.tile_pool(name="aggp", bufs=2, space="PSUM") as aggp,
        ):
            ident = cp.tile([P, P], F32)
            make_identity(nc, ident[:])
            iotar_i = cp.tile([P, P], I32)
            nc.gpsimd.iota(iotar_i[:], pattern=[[1, P]], base=0, channel_multiplier=0)
            iotar = cp.tile([P, P], F32)
            nc.vector.tensor_copy(iotar[:], iotar_i[:])
            iotac_i = cp.tile([P, P], I32)
            nc.gpsimd.iota(iotac_i[:], pattern=[[0, P]], base=0, channel_multiplier=1)
            iotac = cp.tile([P, P], F32)
            nc.vector.tensor_copy(iotac[:], iotac_i[:])
            meae_t = cp.tile([128, D_ROW], F32)
            nc.sync.dma_start(out=meae_t[:], in_=meae[:])

            for s in range(S):
                r0 = s * P
                agg = aggp.tile([P, D_ROW], F32, tag="agg")
                tdst_t = stp.tile([P, D_ROW], F32, tag="tdst")
                nc.sync.dma_start(out=tdst_t[:], in_=tdst[r0:r0 + P, :])
                e_st = stp.tile([P, B], F32, tag="e_st")
                nc.sync.dma_start(out=e_st[:], in_=e_sp[r0:r0 + P, :])
                si_all = stp.tile([P, BT], I32, tag="si_all")
                nc.sync.dma_start(out=si_all[:], in_=srci[r0:r0 + P, :])
                dri_all = stp.tile([P, BT], I32, tag="dri_all")
                nc.sync.dma_start(out=dri_all[:], in_=dstr[r0:r0 + P, :])
                dr_all = stp.tile([P, BT], F32, tag="dr_all")
                nc.vector.tensor_copy(dr_all[:], dri_all[:])
                eo_st = stp.tile([P, B], F32, tag="eo_st")
                e_m1 = stp.tile([P, B], F32, tag="e_m1")
                nc.vector.tensor_scalar_add(e_m1[:], e_st[:], -1.0)

                for bt in range(BT):
                    ec = slice(bt * P, (bt + 1) * P)
                    srow = sb.tile([P, D_SROW], F32, tag="srow")
                    nc.gpsimd.indirect_dma_start(
                        out=srow[:], out_offset=None, in_=tsrc[:],
                        in_offset=bass.IndirectOffsetOnAxis(
                            ap=si_all[:, bt:bt + 1], axis=0))

                    # sel[e,n] / selT[n,e] equality matrices
                    dcol = dr_all[:, bt:bt + 1]
                    dT_ps = ps.tile([P, P], F32, tag="dTp")
                    nc.tensor.transpose(out=dT_ps[:], in_=dcol.to_broadcast([P, P]),
                                        identity=ident[:])
                    selT = sb.tile([P, P], F32, tag="selT")
                    nc.vector.tensor_tensor(out=selT[:], in0=dT_ps[:], in1=iotac[:], op=EQ)
                    sel = sb.tile([P, P], F32, tag="sel")
                    nc.vector.tensor_tensor(out=sel[:], in0=dcol.to_broadcast([P, P]),
                                            in1=iotar[:], op=EQ)

                    # pa = e@[M_e|A_e] + sel@tdst + srow[:,0:136], all in PSUM
                    eT_ps = ps.tile([P, 128], F32, tag="eTp")
                    nc.tensor.transpose(out=eT_ps[:], in_=e_st[:, ec], identity=ident[:])
                    eT = sb.tile([P, 128], F32, tag="eT")
                    nc.vector.tensor_copy(eT[:], eT_ps[:])
                    pa_ps = ps.tile([P, D_ROW], F32, tag="pap")
                    nc.tensor.matmul(pa_ps[:], eT[:], meae_t[:], start=True, stop=False)
                    nc.tensor.matmul(pa_ps[:], selT[:], tdst_t[:], start=False, stop=False)
                    nc.tensor.matmul(pa_ps[:], ident[:], srow[:, 0:D_ROW],
                                     start=False, stop=True)

                    # ex = exp(leaky_relu(attn)) into rhs[:,128:136]
                    rhs = sb.tile([P, D_ROW], F32, tag="rhs")
                    lk = sb.tile([P, H], F32, tag="lk")
                    nc.vector.tensor_scalar_mul(lk[:], pa_ps[:, 128:136], 0.01)
                    lk2 = sb.tile([P, H], F32, tag="lk2")
                    nc.vector.tensor_tensor(out=lk2[:], in0=pa_ps[:, 128:136],
                                            in1=lk[:], op=MX)
                    nc.scalar.activation(rhs[:, 128:136], lk2[:], EXP)

                    # e_out slice = e + elu(e_proj); elu(x) = exp(-relu(-x)) - 1 + relu(x)
                    rn = sb.tile([P, 128], F32, tag="rn")
                    nc.scalar.activation(rn[:], pa_ps[:, :128], RELU, scale=-1.0)
                    exel = sb.tile([P, 128], F32, tag="exel")
                    nc.scalar.activation(exel[:], rn[:], EXP, scale=-1.0)
                    rp = sb.tile([P, 128], F32, tag="rp")
                    nc.scalar.activation(rp[:], pa_ps[:, :128], RELU)
                    d1 = sb.tile([P, 128], F32, tag="d1")
                    nc.vector.tensor_tensor(out=d1[:], in0=exel[:], in1=rp[:], op=AD)
                    nc.vector.tensor_tensor(out=eo_st[:, ec], in0=d1[:],
                                            in1=e_m1[:, ec], op=AD)

                    # rhs[:,0:128] = z * ex (broadcast ex over each head's 16 cols)
                    nc.vector.tensor_tensor(
                        out=rhs[:, 0:128].rearrange("p (h o) -> p h o", h=H),
                        in0=srow[:, D_ROW:D_SROW].rearrange("p (h o) -> p h o", h=H),
                        in1=rhs[:, 128:136].to_broadcast([P, H, O]),
                        op=mybir.AluOpType.mult)
                    # agg += sel.T @ [z*ex | ex]
                    nc.tensor.matmul(agg[:], sel[:], rhs[:],
                                     start=(bt == 0), stop=(bt == BT - 1))

                nc.sync.dma_start(out=eo[r0:r0 + P, :], in_=eo_st[:])

                # stripe flush: h_out = h + elu(agg/denom)
                h_t = sb.tile([P, 128], F32, tag="h_t")
                nc.sync.dma_start(out=h_t[:], in_=hsh[r0:r0 + P, :])
                den = sb.tile([P, H], F32, tag="den")
                nc.vector.tensor_scalar_max(den[:], agg[:, 128:136], 1e-9)
                rec = sb.tile([P, H], F32, tag="rec")
                nc.vector.reciprocal(rec[:], den[:])
                hag = sb.tile([P, 128], F32, tag="hag")
                nc.vector.tensor_tensor(
                    out=hag[:].rearrange("p (h o) -> p h o", h=H),
                    in0=agg[:, 0:128].rearrange("p (h o) -> p h o", h=H),
                    in1=rec[:].to_broadcast([P, H, O]),
                    op=mybir.AluOpType.mult)
                rn2 = sb.tile([P, 128], F32, tag="rn2")
                nc.scalar.activation(rn2[:], hag[:], RELU, scale=-1.0)
                exel2 = sb.tile([P, 128], F32, tag="exel2")
                nc.scalar.activation(exel2[:], rn2[:], EXP, scale=-1.0)
                rp2 = sb.tile([P, 128], F32, tag="rp2")
                nc.scalar.activation(rp2[:], hag[:], RELU)
                el2 = sb.tile([P, 128], F32, tag="el2")
                nc.vector.tensor_tensor(out=el2[:], in0=exel2[:], in1=rp2[:], op=AD)
                d3 = sb.tile([P, 128], F32, tag="d3")
                nc.vector.tensor_tensor(out=d3[:], in0=el2[:], in1=h_t[:], op=AD)
                ho_t = sb.tile([P, 128], F32, tag="ho_t")
                nc.vector.tensor_scalar_add(ho_t[:], d3[:], -1.0)
                nc.sync.dma_start(out=ho[r0:r0 + P, :], in_=ho_t[:])

    nc.compile()
    _NC_CACHE[key] = nc
    return nc


def prep(h, e, src, dst, Wh, We, Wproj, bproj, Wattn, n_cores=N_CORES):
    """Host-side fold + sort + shard. Returns (in_maps, meta)."""
    N, E = h.shape[0], e.shape[0]
    h = np.ascontiguousarray(np.asarray(h, np.float32))
    e = np.ascontiguousarray(np.asarray(e, np.float32))
    src = np.asarray(src, np.int32); dst = np.asarray(dst, np.int32)
    Wh = np.asarray(Wh, np.float32); We = np.asarray(We, np.float32)
    Wproj = np.asarray(Wproj, np.float32); bproj = np.asarray(bproj, np.float32)
    Wattn = np.asarray(Wattn, np.float32)

    S = math.ceil(N / (P * n_cores))          # stripes per core
    nstr = S * n_cores
    NPAD = nstr * P

    # ---- weight folding + node tables ----
    z = (h @ Wh.transpose(1, 0, 2).reshape(128, 128)).reshape(N, H, O)
    M_e = np.einsum('hik,hko->iho', We, Wproj[:, :O, :]).reshape(128, 128)
    A_e = np.einsum('hik,hk->ih', We, Wattn[:, :O])
    meae = np.concatenate([M_e, A_e], axis=1).astype(np.float32)
    p_src = np.einsum('nhk,hko->nho', z, Wproj[:, O:2 * O, :]).reshape(N, 128)
    a_src = np.einsum('nhk,hk->nh', z, Wattn[:, O:2 * O])
    p_dst = (np.einsum('nhk,hko->nho', z, Wproj[:, 2 * O:, :]) + bproj).reshape(N, 128)
    a_dst = np.einsum('nhk,hk->nh', z, Wattn[:, 2 * O:])
    tsrc = np.zeros((NPAD, D_SROW), np.float32)
    tsrc[:N] = np.concatenate([p_src, a_src, z.reshape(N, 128)], axis=1)
    tdst_full = np.zeros((NPAD, D_ROW), np.float32)
    tdst_full[:N] = np.concatenate([p_dst, a_dst], axis=1)
    hpad = np.zeros((NPAD, 128), np.float32)
    hpad[:N] = h

    # ---- sort edges by dst, stripe-pad to uniform B = BT*128 ----
    perm = np.argsort(dst, kind='stable').astype(np.int64)
    dst_s = dst[perm]
    starts = np.searchsorted(dst_s, np.arange(nstr) * P).astype(np.int64)
    ends = np.searchsorted(dst_s, np.arange(nstr) * P + P).astype(np.int64)
    cnt = ends - starts
    BT = max(1, math.ceil(cnt.max() / P))
    B = BT * P
    slot = np.arange(B)
    mat = starts[:, None] + slot[None, :]
    valid = slot[None, :] < cnt[:, None]                   # [nstr, B]
    spos = np.where(valid, mat, 0)
    eidx = perm[spos]                                      # original edge ids
    eidx_g = np.where(valid, eidx, 0)
    src_pad = np.where(valid, src[eidx_g], 0).astype(np.int32)
    dstrel_pad = np.where(valid, dst[eidx_g] - (np.arange(nstr) * P)[:, None],
                          255).astype(np.int32)
    e_sp = e[eidx_g.reshape(-1)]
    # block-transpose: [nstr, BT, 128e, 128k] -> [nstr*128e, BT*128k]
    e_sp = np.ascontiguousarray(
        e_sp.reshape(nstr, BT, P, 128).transpose(0, 2, 1, 3).reshape(nstr * P, B))
    src_pad = np.ascontiguousarray(
        src_pad.reshape(nstr, BT, P).transpose(0, 2, 1).reshape(nstr * P, BT))
    dstrel_pad = np.ascontiguousarray(
        dstrel_pad.reshape(nstr, BT, P).transpose(0, 2, 1).reshape(nstr * P, BT))

    in_maps = []
    for c in range(n_cores):
        lo, hi = c * S * P, (c + 1) * S * P
        in_maps.append({
            "e_sp": e_sp[lo:hi],
            "tsrc": tsrc,
            "tdst": tdst_full[lo:hi],
            "hsh": hpad[lo:hi],
            "srci": src_pad[lo:hi],
            "dstr": dstrel_pad[lo:hi],
            "meae": meae,
        })
    meta = dict(S=S, BT=BT, NPAD=NPAD, N=N, E=E, n_cores=n_cores,
                eidx=eidx, valid=valid)
    return in_maps, meta


def unshard(results, meta):
    N, E, BT = meta["N"], meta["E"], meta["BT"]
    nstr = meta["NPAD"] // P
    ho = np.concatenate([r["ho"] for r in results], axis=0)[:N]
    eo = np.concatenate([r["eo"] for r in results], axis=0)
    eo = eo.reshape(nstr, P, BT, 128).transpose(0, 2, 1, 3).reshape(nstr, BT * P, 128)
    e_out = np.empty((E, 128), np.float32)
    v = meta["valid"]
    e_out[meta["eidx"][v]] = eo[v]
    return np.ascontiguousarray(ho), e_out


def kernel(h, e, src, dst, Wh, We, Wproj, bproj, Wattn):
    in_maps, meta = prep(h, e, src, dst, Wh, We, Wproj, bproj, Wattn)
    nc = build_nc(meta["S"], meta["BT"], meta["NPAD"])
    res = run_bass_kernel_spmd(nc, in_maps, core_ids=list(range(N_CORES)))
    h_out, e_out = unshard(res.results, meta)
    return h_out, e_out
